# revision 15
# baseline (speedup 1.0000x reference)
"""Trainium2 Bass kernel for a single-layer decoder-only transformer.

Problem shapes: B=2, S=2048, D=1024, M=4096, OMEGA=32000 (fp32 reference).

Sharding: 8 cores, token-parallel.  Core c = (b, j) with b = c//4 owns the
STRIDED query set {4p + j : p in [0, 512)} of batch b.  The stride-4
assignment balances causal attention exactly: query group m (128 queries)
only attends to keys < 512*(m+1) on EVERY core, so all cores skip the same
37.5% of score/softmax work while running one identical SPMD program.

  A. gather emb rows for the 512 strided queries -> hq^T; Q^T = Wq^T hq^T.
  B+C1 pipelined over 4 context chunks of 512 tokens: embed chunk ->
     K^T chunk (SBUF-resident, no DRAM spill) + V chunk -> scores for all
     query groups m >= chunk; softmax(m=chunk) overlaps the next chunk.
  C2/C3: attn = P V, Wo projection, gelu FFN, all feature-major.
  D. logits = ffT^T @ Wl + bl streamed over the 32000-col vocab; each core
     writes a [512, 32000] fp32 slab (rows = strided queries).

All matmuls run in bf16 (weights + embedding rounded host-side) with fp32
PSUM accumulation; end-to-end error vs the fp32 reference ~0.5% relative.
"""

import numpy as np
import ml_dtypes

import concourse.bass as bass
import concourse.bacc as bacc
import concourse.tile as tile
import concourse.mybir as mybir
from concourse.bass_utils import run_bass_kernel_spmd
from concourse.masks import make_identity

P = 128
B, S, D, M, V = 2, 2048, 1024, 4096, 32000
TQ = 512              # queries per core (strided by 4 over the batch)
CTX = S               # context length (uniform across cores)
NCORES = 8
GRP = NCORES // B     # 4 cores per batch
DC = D // P           # 8 feature chunks
MC = M // P           # 32 ffn chunks
KC = CTX // P         # 16 key chunks (128 keys each)
QS = TQ // P          # 4 query subtiles
CCH = CTX // 512      # 4 context chunks for the A/B/C1 pipeline
N_TILE = 512
N_TILES = [(i * N_TILE, min(N_TILE, V - i * N_TILE)) for i in range((V + N_TILE - 1) // N_TILE)]

F32 = mybir.dt.float32
BF16 = mybir.dt.bfloat16
I32 = mybir.dt.int32
I16 = mybir.dt.int16
AF = mybir.ActivationFunctionType
AX = mybir.AxisListType

_CACHE = {}


def _bcast_ap(t, offset, n, length):
    """DRAM AP broadcasting a [length] row to [n, length] partitions."""
    return bass.AP(tensor=t.tensor, offset=offset, ap=[[0, n], [1, length]])


def build_program():
    nc = bacc.Bacc("TRN2", target_bir_lowering=False, debug=False,
                   num_devices=NCORES)

    def din(name, shape, dt):
        return nc.dram_tensor(name, shape, dt, kind="ExternalInput").ap()

    xk = din("xk", [P, CTX // 16], I16)   # wrapped idx (i%16, i//16), x8 replicated
    xq = din("xq", [P, TQ // 16], I16)
    emb = din("emb", [V, D], BF16)
    pek = din("pek", [CCH, P, DC * 512], BF16)   # pe^T per 512-token chunk
    peq = din("peq", [P, DC * TQ], BF16)
    wq = din("wq", [D, D], BF16)
    wk = din("wk", [D, D], BF16)
    wv = din("wv", [D, D], BF16)
    wo = din("wo", [D, D], BF16)
    wf = din("wf", [D, M], BF16)
    wl = din("wl", [M, V], BF16)
    bq = din("bq", [D], F32)
    bk = din("bk", [D], F32)
    bv = din("bv", [D], F32)
    bo = din("bo", [D], F32)
    bf_ = din("bf", [M], F32)
    bl = din("bl", [V], F32)
    maskq = din("maskq", [QS, P, CTX], BF16)
    out = nc.dram_tensor("out", [TQ, V], BF16, kind="ExternalOutput").ap()

    wq_r = wq.rearrange("(c p) o -> p c o", p=P)
    wk_r = wk.rearrange("(c p) o -> p c o", p=P)
    wv_r = wv.rearrange("(c p) o -> p c o", p=P)
    wo_r = wo.rearrange("(c p) o -> p c o", p=P)
    wf_r = wf.rearrange("(c p) o -> p c o", p=P)
    wl_r = wl.rearrange("(c p) o -> p c o", p=P)

    with tile.TileContext(nc) as tc:
        _emit(nc, tc, locals())
    nc.compile()
    return nc


def _emit(nc, tc, t):
    import contextlib
    ctx = contextlib.ExitStack()
    with ctx:
        main = ctx.enter_context(tc.tile_pool(name="main", bufs=1))
        stC = ctx.enter_context(contextlib.ExitStack())
        poolC = stC.enter_context(tc.tile_pool(name="poolC", bufs=1))
        poolW = stC.enter_context(tc.tile_pool(name="poolW", bufs=2))
        stAB = ctx.enter_context(contextlib.ExitStack())
        poolAB = stAB.enter_context(tc.tile_pool(name="poolAB", bufs=1))
        stSP = stAB.enter_context(contextlib.ExitStack())
        poolSP = stSP.enter_context(tc.tile_pool(name="poolSP", bufs=1))
        # PSUM pools (own LIFO stack)
        stPB = ctx.enter_context(contextlib.ExitStack())
        psB = stPB.enter_context(tc.tile_pool(name="psB", bufs=3, space="PSUM"))
        stPA = stPB.enter_context(contextlib.ExitStack())
        psA = stPA.enter_context(tc.tile_pool(name="psA", bufs=2, space="PSUM"))

        ident = main.tile([P, P], BF16, tag="ident")

        # ---- biases (loaded lazily, after the first gathers are queued) ----
        bqt = main.tile([P, DC], F32, tag="bqt")
        bkt = main.tile([P, DC], F32, tag="bkt")
        bot = main.tile([P, DC], F32, tag="bot")
        bft = main.tile([P, MC], F32, tag="bft")
        bvb = main.tile([P, D], BF16, tag="bvb")

        def load_biases():
            nc.sync.dma_start(out=bqt[:],
                              in_=t["bq"].rearrange("(c p) -> p c", p=P))
            nc.sync.dma_start(out=bkt[:],
                              in_=t["bk"].rearrange("(c p) -> p c", p=P))
            nc.sync.dma_start(out=bot[:],
                              in_=t["bo"].rearrange("(c p) -> p c", p=P))
            nc.sync.dma_start(out=bft[:],
                              in_=t["bf_"].rearrange("(c p) -> p c", p=P))
            nc.gpsimd.dma_start(out=bvb[:], in_=_bcast_ap(t["bv"], 0, P, D))

        hqT = poolAB.tile([P, DC, TQ], BF16, tag="hqT")   # hq^T [D, TQ]
        qT = poolAB.tile([P, DC, TQ], BF16, tag="qT")     # Q^T  [D, TQ]
        kT = poolAB.tile([P, DC, CTX], BF16, tag="kT")    # K^T  [D, CTX]
        vtok = poolAB.tile([P, KC, D], BF16, tag="vtok")  # V    [CTX, D]
        # scores, exact causal width per query group
        sC = [poolSP.tile([P, 512 * (m + 1)], BF16, tag=f"sC{m}",
                          name=f"sC{m}") for m in range(QS)]
        pT = poolSP.tile([P, KC, TQ], BF16, tag="pT")     # P^T  [CTX, TQ]
        atT = poolC.tile([P, DC, TQ], BF16, tag="atT")    # attn^T
        aoT = poolC.tile([P, DC, TQ], BF16, tag="aoT")    # (attn Wo)^T

        def embed_chunk(pep, idxt, s0, peT_ap, dstT_ap):
            """Gather 512 token rows feature-major (xbar transpose) + pe add.

            dma_gather(transpose=True) writes dst[p, c, i] = emb[id_i,
            c*128+p], exactly the h^T layout the projections consume.  idxt
            holds the wrapped ids (i%16, i//16) replicated over the 8 Q7
            cores; column slice [s0/16, (s0+512)/16) selects this chunk's 512
            tokens.  pe^T is then added in place on DVE.
            """
            nc.gpsimd.dma_gather(
                out_ap=dstT_ap, in_ap=t["emb"][:, :],
                idxs_ap=idxt[:, s0 // 16:(s0 + 512) // 16],
                num_idxs=512, num_idxs_reg=512, elem_size=D, transpose=True)
            pet = pep.tile([P, DC, 512], BF16, tag="pet")
            nc.sync.dma_start(
                out=pet[:], in_=peT_ap.rearrange("p (c n) -> p c n", c=DC))
            nc.vector.tensor_add(dstT_ap, dstT_ap, pet[:])

        def load_w_half(wr, h, tag):
            w_ = poolW.tile([P, DC // 2, D], BF16, tag=tag)
            nc.scalar.dma_start(out=w_[:], in_=wr[:, h * 4:(h + 1) * 4, :])
            return w_

        def acc_halves(ps_ap, halves, col, rhs_fn):
            for h in (0, 1):
                for dj in range(4):
                    di = h * 4 + dj
                    nc.tensor.matmul(out=ps_ap,
                                     lhsT=halves[h][:, dj, col],
                                     rhs=rhs_fn(di),
                                     start=(di == 0), stop=(di == DC - 1))

        inv_sqrt_d = 1.0 / float(np.sqrt(D))

        def softmax_group(sm, m):
            """Softmax over the causal key range of query group m, then
            transpose the live key chunks into pT."""
            width = 512 * (m + 1)
            sm_ = sC[m]
            mx = sm.tile([P, 1], F32, tag="mx")
            nc.vector.reduce_max(mx[:], sm_[:], axis=AX.X)
            negmx = sm.tile([P, 1], F32, tag="negmx")
            nc.scalar.mul(negmx[:], mx[:], -inv_sqrt_d)
            mask = sm.tile([P, CTX], BF16, tag="mask", bufs=1)
            nc.sync.dma_start(out=mask[:, :width],
                              in_=t["maskq"][m, :, :width])
            nc.vector.tensor_add(sm_[:], sm_[:], mask[:, :width])
            den = sm.tile([P, 1], F32, tag="den")
            nc.scalar.activation(sm_[:], sm_[:], AF.Exp,
                                 bias=negmx[:, :1], scale=inv_sqrt_d,
                                 accum_out=den[:, :1])
            rden = sm.tile([P, 1], F32, tag="rden")
            nc.vector.reciprocal(rden[:], den[:])
            nc.vector.tensor_scalar_mul(sm_[:], sm_[:], rden[:, :1])
            for kc in range(4 * (m + 1)):
                pt_ = psA.tile([P, P], BF16, tag="psT", space="PSUM")
                nc.tensor.transpose(out=pt_[:],
                                    in_=sm_[:, kc * P:(kc + 1) * P],
                                    identity=ident[:])
                nc.scalar.activation(pT[:, kc, m * P:(m + 1) * P], pt_[:],
                                     AF.Copy)

        # ---- stage A: query embed + Q projection ----
        with nc.named_scope("embed"), tc.tile_pool(name="pep", bufs=2) as pep, \
             tc.tile_pool(name="sm", bufs=2) as sm, \
             tc.tile_pool(name="hck", bufs=2) as hck, \
             tc.tile_pool(name="psS", bufs=3, space="PSUM") as psS:
            idxq = main.tile([P, TQ // 16], I16, tag="idxq")
            idxk = main.tile([P, CTX // 16], I16, tag="idxk")
            with tc.high_priority():
                nc.sync.dma_start(out=idxq[:], in_=t["xq"])
                nc.sync.dma_start(out=idxk[:], in_=t["xk"])
                embed_chunk(pep, idxq, 0, t["peq"], hqT[:, :, :])
            make_identity(nc, ident[:])
            load_biases()
            wq_h = [load_w_half(t["wq_r"], h, "wk_h") for h in (0, 1)]
            for dc in range(DC):
                ps = psB.tile([P, N_TILE], F32, tag="psB", space="PSUM")
                acc_halves(ps[:, :TQ], wq_h, slice(dc * P, (dc + 1) * P),
                           lambda di: hqT[:, di, :])
                nc.scalar.activation(qT[:, dc, :], ps[:, :TQ], AF.Identity,
                                     bias=bqt[:, dc:dc + 1])

            # ---- stage B+C1: pipelined context chunks ----
            wk_h = [load_w_half(t["wk_r"], h, "wk_h") for h in (0, 1)]
            wv_h = [load_w_half(t["wv_r"], h, "wv_h") for h in (0, 1)]
            for c4 in range(CCH):
                sl = slice(c4 * 512, (c4 + 1) * 512)
                hTc = hck.tile([P, DC, 512], BF16, tag="hTc")
                embed_chunk(pep, idxk, c4 * 512, t["pek"][c4, :, :],
                            hTc[:, :, :])
                # K^T chunk (SBUF resident)
                for dc in range(DC):
                    ps = psB.tile([P, N_TILE], F32, tag="psB", space="PSUM")
                    acc_halves(ps[:], wk_h, slice(dc * P, (dc + 1) * P),
                               lambda di: hTc[:, di, :])
                    nc.scalar.activation(kT[:, dc, sl], ps[:], AF.Identity,
                                         bias=bkt[:, dc:dc + 1])
                # softmax of the group completed last chunk: its serial
                # vector chain overlaps this chunk's K matmuls, so the pT
                # transposes queued here reach the PE with no stall.
                if c4 > 0:
                    softmax_group(sm, c4 - 1)
                # V chunk (token-major)
                for tc4 in range(4):
                    kc = c4 * 4 + tc4
                    for nn in range(D // N_TILE):
                        nsl = slice(nn * N_TILE, (nn + 1) * N_TILE)
                        ps = psB.tile([P, N_TILE], F32, tag="psB", space="PSUM")
                        for h in (0, 1):
                            for dj in range(4):
                                di = h * 4 + dj
                                nc.tensor.matmul(
                                    out=ps[:],
                                    lhsT=hTc[:, di, tc4 * P:(tc4 + 1) * P],
                                    rhs=wv_h[h][:, dj, nsl],
                                    start=(di == 0), stop=(di == DC - 1))
                        nc.vector.tensor_add(vtok[:, kc, nsl], ps[:],
                                             bvb[:, nsl])
                # scores vs this key chunk for all live query groups; the
                # group whose causal range ends here (m == c4) goes last and
                # its softmax overlaps the next chunk's embed + K/V.
                for m in range(QS - 1, c4 - 1, -1):
                    ps = psS.tile([P, N_TILE], F32, tag="psS", space="PSUM")
                    for di in range(DC):
                        nc.tensor.matmul(out=ps[:],
                                         lhsT=qT[:, di, m * P:(m + 1) * P],
                                         rhs=kT[:, di, sl],
                                         start=(di == 0), stop=(di == DC - 1))
                    nc.scalar.activation(sC[m][:, sl], ps[:], AF.Copy)

            # ---- stage C2: attn^T = V-blocks^T @ P^T ----
            # key chunk kc only feeds query groups m >= kc//4; pT is never
            # written (and P is exactly 0) outside that range, so each
            # accumulation step narrows to the live query columns.  Query
            # groups 0..2 (C2a) don't need the last softmax, so they run
            # under group 3's softmax chain; group 3's columns (C2b) follow.
            for dc in range(DC):
                ps = psB.tile([P, N_TILE], F32, tag="psB", space="PSUM")
                for kc in range(12):
                    q0 = (kc // 4) * P
                    nc.tensor.matmul(out=ps[:, q0:3 * P],
                                     lhsT=vtok[:, kc, dc * P:(dc + 1) * P],
                                     rhs=pT[:, kc, q0:3 * P],
                                     start=(kc == 0), stop=(kc == 11),
                                     skip_group_check=True)
                nc.scalar.activation(atT[:, dc, :3 * P], ps[:, :3 * P], AF.Copy)
            softmax_group(sm, CCH - 1)
            for dc in range(DC):
                ps = psB.tile([P, N_TILE], F32, tag="psB", space="PSUM")
                for kc in range(KC):
                    nc.tensor.matmul(out=ps[:, :P],
                                     lhsT=vtok[:, kc, dc * P:(dc + 1) * P],
                                     rhs=pT[:, kc, 3 * P:TQ],
                                     start=(kc == 0), stop=(kc == KC - 1))
                nc.scalar.activation(atT[:, dc, 3 * P:], ps[:, :P], AF.Copy)
        stPA.close()  # transpose psum done after C1

        with nc.named_scope("attn"):
            stSP.close()  # sC/pT dead after C2
            stAB.close()  # hqT/qT/kT/vtok dead after C2

            # stage-D pools open here so Wl slab prefetch can fill the
            # C3a/C3b DMA-idle window (space freed by poolAB)
            wlp = ctx.enter_context(tc.tile_pool(name="wlp", bufs=1,
                                                 side="right"))
            blp = ctx.enter_context(tc.tile_pool(name="blp", bufs=2,
                                                 side="right"))
            outp = ctx.enter_context(tc.tile_pool(name="outp", bufs=4,
                                                  side="right"))

            # ---- stage C3a: attnout^T = Wo^T @ attn^T ----
            wo_h = [load_w_half(t["wo_r"], h, "wv_h") for h in (0, 1)]
            for dc in range(DC):
                ps = psB.tile([P, N_TILE], F32, tag="psB", space="PSUM")
                acc_halves(ps[:, :TQ], wo_h, slice(dc * P, (dc + 1) * P),
                           lambda di: atT[:, di, :])
                nc.scalar.activation(aoT[:, dc, :], ps[:, :TQ], AF.Identity,
                                     bias=bot[:, dc:dc + 1])

        # ---- stage C3b: ffT = gelu(Wf^T @ aoT + bf) ----
        poolFF = ctx.enter_context(tc.tile_pool(name="poolFF", bufs=1,
                                                side="right"))
        ffT = poolFF.tile([P, MC, TQ], BF16, tag="ffT")   # ff^T [M, TQ]
        with nc.named_scope("ffn"), tc.tile_pool(name="poolWF", bufs=2) as poolWF:
            # wf streamed in M-column quarters: quarter q serves mc 8q..8q+7
            MQ = MC // 4
            for q in range(4):
                w_ = poolWF.tile([P, DC, MQ * P], BF16, tag="w_f")
                nc.scalar.dma_start(
                    out=w_[:], in_=t["wf_r"][:, :, q * MQ * P:(q + 1) * MQ * P])
                for mj in range(MQ):
                    mc = q * MQ + mj
                    ps = psB.tile([P, N_TILE], F32, tag="psB", space="PSUM")
                    for di in range(DC):
                        nc.tensor.matmul(out=ps[:, :TQ],
                                         lhsT=w_[:, di, mj * P:(mj + 1) * P],
                                         rhs=aoT[:, di, :],
                                         start=(di == 0), stop=(di == DC - 1))
                    nc.scalar.activation(ffT[:, mc, :], ps[:, :TQ], AF.Gelu,
                                         bias=bft[:, mc:mc + 1])
        stPB.close()
        stC.close()  # atT/aoT + weight-half pools dead after FFN

        # ---- stage D: logits = ffT^T @ Wl + bl ----
        with nc.named_scope("logits"), \
             tc.tile_pool(name="psD", bufs=6, space="PSUM") as psD:
            for (n0, nsz) in N_TILES:
                slab = wlp.tile([P, MC, N_TILE], BF16, tag="slab", bufs=2)
                for qq in range(4):
                    nc.sync.dma_start(
                        out=slab[:, qq * 8:(qq + 1) * 8, :nsz],
                        in_=t["wl_r"][:, qq * 8:(qq + 1) * 8, n0:n0 + nsz])
                blt = blp.tile([P, N_TILE], F32, tag="blt")
                nc.gpsimd.dma_start(out=blt[:, :nsz],
                                    in_=_bcast_ap(t["bl"], n0, P, nsz))
                for m in range(QS):
                    ps = psD.tile([P, N_TILE], F32, tag="psD", space="PSUM")
                    for kc in range(MC):
                        nc.tensor.matmul(out=ps[:, :nsz],
                                         lhsT=ffT[:, kc, m * P:(m + 1) * P],
                                         rhs=slab[:, kc, :nsz],
                                         start=(kc == 0), stop=(kc == MC - 1))
                    ot = outp.tile([P, N_TILE], BF16, tag="ot")
                    nc.vector.tensor_add(ot[:, :nsz], ps[:, :nsz], blt[:, :nsz])
                    eng = nc.sync if m % 2 == 0 else nc.scalar
                    eng.dma_start(
                        out=t["out"][m * P:(m + 1) * P, n0:n0 + nsz],
                        in_=ot[:, :nsz])


def _wrap_ids(ids):
    """int16 idx layout for dma_gather: (i%16, i//16), x8 over 128 parts."""
    n = ids.shape[0]
    w = np.asarray(ids, np.int16).reshape(n // 16, 16).T  # [16, n//16]
    return np.ascontiguousarray(np.tile(w, (8, 1)))


def _pe_T(pe32, rows, bf16):
    """pe rows -> feature-major [128, DC*len(rows)]: out[p, c*n+t] =
    pe[rows[t], c*128+p]."""
    sel = pe32[rows]                               # [n, D]
    return np.ascontiguousarray(
        sel.reshape(-1, DC, P).transpose(2, 1, 0)  # [P, DC, n]
        .reshape(P, -1).astype(bf16))


def _prep_inputs(x, emb, pe, Wq, bq, Wk, bk, Wv, bv, Wo, bo, Wf, bf, Wl, bl):
    """Host-side sharding / layout prep (no data-dependent compute)."""
    bf16 = ml_dtypes.bfloat16
    x = np.asarray(x)
    pe32 = np.asarray(pe, np.float32)[:CTX]
    pekT = np.stack([_pe_T(pe32, np.arange(c * 512, (c + 1) * 512), bf16)
                     for c in range(CCH)])         # [CCH, P, DC*512]
    shared = {
        "emb": np.ascontiguousarray(np.asarray(emb).astype(bf16)),
        "pek": pekT,
        "wq": np.asarray(Wq).astype(bf16),
        "wk": np.asarray(Wk).astype(bf16),
        "wv": np.asarray(Wv).astype(bf16),
        "wo": np.asarray(Wo).astype(bf16),
        "wf": np.asarray(Wf).astype(bf16),
        "wl": np.ascontiguousarray(np.asarray(Wl).astype(bf16)),
        "bq": np.asarray(bq, np.float32),
        "bk": np.asarray(bk, np.float32),
        "bv": np.asarray(bv, np.float32),
        "bo": np.asarray(bo, np.float32),
        "bf": np.asarray(bf, np.float32),
        "bl": np.asarray(bl, np.float32),
    }
    in_maps = []
    for c in range(NCORES):
        b, j = divmod(c, GRP)
        # strided queries: global row of local query p is 4p + j
        gq_all = 4 * np.arange(TQ) + j
        mask = np.zeros((QS, P, CTX), dtype=bf16)
        for m in range(QS):
            gq = gq_all[m * P:(m + 1) * P][:, None]
            vis = np.arange(CTX)[None, :] <= gq
            mask[m] = np.where(vis, np.float32(0.0), np.float32(-1e9)).astype(bf16)
        im = dict(shared)
        im["xk"] = _wrap_ids(x[b])
        im["xq"] = _wrap_ids(x[b, gq_all])
        im["peq"] = _pe_T(pe32, gq_all, bf16)
        im["maskq"] = mask
        in_maps.append(im)
    return in_maps


def kernel(**inputs):
    if "nc" not in _CACHE:
        _CACHE["nc"] = build_program()
    nc = _CACHE["nc"]
    in_maps = _prep_inputs(**inputs)
    res = run_bass_kernel_spmd(nc, in_maps, list(range(NCORES)))
    x = np.asarray(inputs["x"])
    Bsz, Ssz = x.shape
    out = np.empty((Bsz, Ssz, V), np.float32)
    for c in range(NCORES):
        b, j = divmod(c, GRP)
        out[b, j::4] = res.results[c]["out"]
    return out


if __name__ == "__main__":
    pass

